# revision 24
# baseline (speedup 1.0000x reference)
"""HSMNet cost-volume + disparity softmax-regression on 8 Trainium2 NeuronCores.

Reference computation (per batch b):
  cost[c,d,h,w] = |ref[c,h,w] - tgt[c,h,w-d]| for w>=d else 0
  cost_agg[d,h,w] = sum_c cost
  pred[h,w] = sum_d d * softmax_d(cost_agg)

Sharding: 8 cores = 4 batches x 2 h-halves (40 rows of 80 each). Each core
processes its [32, 40, 160] slice fully fused on-chip.

Host prep (layout only, no arithmetic): inputs are cast to fp16 and
replicated into 4 partition groups (partition = c + 32*j) with the shift j
baked into tgt via a 24-col front zero pad. On-chip, per eighth of the
pixel range (800 pixels):
  - one DVE tensor_tensor subtract with a 3D access pattern (disparity
    block dim stride +4 on tgt, stride 0 broadcast on ref) produces diffs
    for all 24 disparities: diff[c+32j, k, p] = ref[c,p] - tgt[c, p-4b-j],
    b = 5-k.
  - abs in place, split across DVE (uint16 bitand), ACT (Abs), GPSIMD
    (uint16 bitand) per env-tunable column split.
  - TensorE reduces channels with 0/1 weights into PSUM [24, 2x512], plus
    one extra accumulation matmul that adds -10000 where w < d (validity
    mask folded into the PE pass: [w<d] = sum_k [k<d]*[w==k]).
  - ACT Exp evacuates PSUM -> E[96, 1600] bf16 (rows 24q+d).
  - TensorE contracts E with [ones; d] weights -> den/num [8, 1600].
  - host divides num/den (invalid entries' terms vanish: exp(-1e4) = 0).
"""
import os
import sys
import threading

for _p in ("/opt/trn_rl_repo",):
    if os.path.isdir(_p) and _p not in sys.path:
        sys.path.insert(0, _p)

import numpy as np
import ml_dtypes

import concourse.bacc as bacc
import concourse.mybir as mybir
from concourse.tile import TileContext
from concourse.bass_utils import run_bass_kernel_spmd

dt = mybir.dt

# problem shape (hardcoded per spec)
B, C, H, W = 4, 32, 80, 160
D = 24
HP = H // 2            # rows per core
PIX = HP * W           # 6400 pixels per core
NB = D // 4            # 6 disparity blocks of 4
PAD = 24               # zero pad columns in front of tgtr
NE = 8                 # processing units (eighths of the pixel range)
EW = PIX // NE         # 800 pixels per eighth
QW = PIX // 4          # 1600 pixels per quarter (E column range)
N_CORES = 8

# abs column split within each [128, 4800] diff tile: [0:A) DVE bitand,
# [A:B) ACT Abs, [B:4800) GPSIMD bitand. Multiples of 16.
ABS_DVE = int(os.environ.get("HSM_ABS_DVE", "2560"))
ABS_ACT = int(os.environ.get("HSM_ABS_ACT", "2240"))
DIFF_BUFS = int(os.environ.get("HSM_DIFF_BUFS", "5"))
COST_BUFS = int(os.environ.get("HSM_COST_BUFS", "2"))
OUT_VIA_ACT = int(os.environ.get("HSM_OUT_ACT", "1"))


# packed per-quarter input: cols [0, QW) = ref quarter, [QW, QW+PAD+QW) =
# tgt window [QW*q - PAD, QW*q + QW)
INW = QW + PAD + QW  # 3224
# packed consts (all fp16 container): lred | lnd(bf16 bits) | lmask | maskc
CONST_W = NB * D + 8 + D + EW  # 144+8+24+800 = 976


def _build_program():
    nc = bacc.Bacc("TRN2", target_bir_lowering=False)
    inq_h = nc.dram_tensor("inq", [128, 4 * INW], dt.float16,
                           kind="ExternalInput")
    cst_h = nc.dram_tensor("cst", [128, CONST_W], dt.float16,
                           kind="ExternalInput")
    out_h = nc.dram_tensor("out", [8, 4 * 400], dt.float32, kind="ExternalOutput")

    with TileContext(nc) as tc:
        with tc.tile_pool(name="const", bufs=1) as cpool, \
             tc.tile_pool(name="inp", bufs=4) as ipool, \
             tc.tile_pool(name="diffp", bufs=DIFF_BUFS) as dpool, \
             tc.tile_pool(name="ep", bufs=1) as epool:
            cst_sb = cpool.tile([128, CONST_W], dt.float16)
            lred_sb = cst_sb[:, 0:NB * D]
            lnd_sb = cst_sb[:, NB * D:NB * D + 8].bitcast(dt.bfloat16)
            lmask_sb = cst_sb[0:D, NB * D + 8:NB * D + 8 + D]
            maskc_sb = cst_sb[0:D, NB * D + 8 + D:CONST_W]

            E = epool.tile([128, QW], dt.bfloat16)

            # one packed DMA per quarter (ref + shifted tgt window), spread
            # over the sync/scalar/gpsimd trigger queues so the ~0.65us
            # descriptor-gen per DMA doesn't serialize ahead of quarter 0
            inq = {}

            def emit_load(q, eng):
                t_ = ipool.tile([128, INW], dt.float16, tag="inq",
                                name=f"inq_{q}")
                eng.dma_start(t_[:], inq_h[:, INW * q:INW * (q + 1)])
                inq[q] = t_

            emit_load(0, nc.sync)
            emit_load(1, nc.scalar)
            nc.sync.dma_start(cst_sb[:], cst_h[:])
            # rows 32q+24..32q+31 are never written by the exp evac; zero
            # them once so the num/den matmul sees 0 (their lnd weights are
            # 0, but garbage could be inf/nan)
            nc.gpsimd.memset(E[:], 0.0)
            emit_load(2, nc.gpsimd)
            emit_load(3, nc.gpsimd)

            diffs = {}

            def emit_tt(e):
                q, hh = e // 2, e % 2
                diff = dpool.tile([128, NB * EW], dt.float16, tag="diff",
                                  name=f"diff_{e}")
                out = diff[:].rearrange("p (k x) -> p k x", x=EW)
                in0 = inq[q][:, EW * hh:EW * hh + EW].unsqueeze(
                    1).broadcast_to([128, NB, EW])
                # tgt part starts at col QW; block k reads window cols
                # [QW+EW*hh+4+4k, +EW) -> diff slot k holds block b = 5-k
                t0 = QW + EW * hh + 4
                in1 = inq[q][:, t0:t0 + EW].unsqueeze(
                    1).broadcast_to([128, NB, EW]).copy()
                in1.ap = in1.ap[:1] + (((4, NB)),) + in1.ap[2:]
                nc.vector.tensor_tensor(out, in0, in1,
                                        mybir.AluOpType.subtract)
                diffs[e] = diff

            def emit_abs(e):
                diff = diffs[e]
                if ABS_DVE > 0:
                    du = diff[:, 0:ABS_DVE].bitcast(dt.uint16)
                    nc.vector.tensor_scalar(du, du, 0x7FFF, None,
                                            mybir.AluOpType.bitwise_and)
                if ABS_ACT > 0:
                    s0, s1 = ABS_DVE, ABS_DVE + ABS_ACT
                    nc.scalar.activation(diff[:, s0:s1], diff[:, s0:s1],
                                         mybir.ActivationFunctionType.Abs)
                if ABS_DVE + ABS_ACT < NB * EW:
                    s0 = ABS_DVE + ABS_ACT
                    g = diff[:, s0:NB * EW]
                    nc.gpsimd.scalar_tensor_tensor(
                        g, g, -1.0, g,
                        op0=mybir.AluOpType.mult, op1=mybir.AluOpType.max)

            costs = {}

            def emit_pe(e, qpool):
                diff = diffs[e]
                cost = qpool.tile([D, 1024], dt.float32, tag="cost",
                                  name=f"cost_{e}")
                for k in range(NB):
                    b = NB - 1 - k
                    for cc in range(2):
                        nc.tensor.matmul(
                            cost[:, 512 * cc:512 * cc + 400],
                            lred_sb[:, D * b:D * (b + 1)],
                            diff[:, EW * k + 400 * cc:EW * k + 400 * cc + 400],
                            start=(k == 0), stop=False)
                for cc in range(2):
                    nc.tensor.matmul(
                        cost[:, 512 * cc:512 * cc + 400],
                        lmask_sb[:],
                        maskc_sb[:, 400 * cc:400 * cc + 400],
                        start=False, stop=(cc == 1))
                costs[e] = cost

            def emit_exp(e):
                q, hh = e // 2, e % 2
                cost = costs[e]
                src = cost[:].rearrange("p (k x) -> p k x", x=512)[:, :, 0:400]
                dst = E[32 * q:32 * q + D,
                        EW * hh:EW * (hh + 1)].rearrange(
                            "p (k x) -> p k x", x=400)
                nc.scalar.activation(dst, src, mybir.ActivationFunctionType.Exp)
                del costs[e], diffs[e]

            with tc.tile_pool(name="cost", bufs=COST_BUFS, space="PSUM") as qpool, \
                 tc.tile_pool(name="nd", bufs=1, space="PSUM") as npool:
                nd = npool.tile([8, 2048], dt.float32)

                def emit_nd(half):
                    # partial num/den contraction over quarters 2h, 2h+1
                    # (PE operands cannot start at partition 96, so halves)
                    for cc in range(4):
                        nc.tensor.matmul(
                            nd[:, 512 * cc:512 * cc + 400],
                            lnd_sb[64 * half:64 * half + 64, :],
                            E[64 * half:64 * half + 64,
                              400 * cc:400 * (cc + 1)],
                            start=(half == 0), stop=(half == 1))

                for e in range(NE):
                    emit_tt(e)
                    emit_abs(e)
                    if e >= 1:
                        emit_pe(e - 1, qpool)
                        emit_exp(e - 1)
                        if e == 5:
                            emit_nd(0)
                emit_pe(NE - 1, qpool)
                emit_exp(NE - 1)
                emit_nd(1)

                ndsrc = nd[:].rearrange("p (k x) -> p k x", x=512)[:, :, 0:400]
                out_sb = epool.tile([8, 4 * 400], dt.float32)
                nc.scalar.activation(
                    out_sb[:].rearrange("p (k x) -> p k x", x=400), ndsrc,
                    mybir.ActivationFunctionType.Copy)
                nc.sync.dma_start(out_h[:], out_sb[:])

    nc.compile()
    return nc


def _host_constants():
    """Single packed [128, CONST_W] fp16 tensor: lred | lnd bits | lmask | maskc."""
    cst = np.zeros((128, CONST_W), np.float16)
    # lred: block b sums channels of partition group j into cost row 4b+j
    for b in range(NB):
        for j in range(4):
            for c in range(C):
                cst[c + 32 * j, D * b + 4 * b + j] = 1.0

    # lnd (bf16, stored as raw bits in the fp16 container)
    lnd = np.zeros((128, 8), np.float32)
    for q in range(4):
        for d in range(D):
            lnd[d + 32 * q, q] = 1.0      # den
            lnd[d + 32 * q, 4 + q] = d    # num
    lnd16 = lnd.astype(ml_dtypes.bfloat16).view(np.float16)
    cst[:, NB * D:NB * D + 8] = lnd16

    # bias[d, p] = sum_k lmask[k, d] * maskc[k, p] = -1e4 * [(p mod W) < d]
    o = NB * D + 8
    for k in range(D):
        for d in range(D):
            if k < d:
                cst[k, o + d] = 1.0
    o = NB * D + 8 + D
    for k in range(D):
        cst[k, o + np.arange(EW)[np.arange(EW) % W == k]] = -10000.0
    return cst


_lock = threading.Lock()
_cache = {}


def _get_program():
    with _lock:
        if "nc" not in _cache:
            _cache["nc"] = _build_program()
            _cache["consts"] = _host_constants()
        return _cache["nc"], _cache["consts"]


def _prep_core(ref_s, tgt_s):
    """ref_s, tgt_s: [32, 6400] fp16 -> packed [128, 4*INW] input."""
    inq = np.zeros((128, 4 * INW), np.float16)
    for q in range(4):
        c0 = QW * q
        inq[:, INW * q:INW * q + QW] = np.broadcast_to(
            ref_s[None, :, c0:c0 + QW], (4, C, QW)).reshape(128, QW)
        # tgt window [c0 - PAD, c0 + QW) with per-group shift j baked in:
        # col QW + s holds tgt[c, c0 - PAD + s - j]
        for j in range(4):
            lo = c0 - PAD - j
            src = tgt_s[:, max(lo, 0):c0 + QW - j]
            inq[32 * j:32 * j + 32,
                INW * q + QW + max(lo, 0) - lo:
                INW * q + QW + max(lo, 0) - lo + src.shape[1]] = src
    return inq


def _run(refimg_fea, targetimg_fea, trace=False):
    nc, cst = _get_program()
    ref = np.asarray(refimg_fea, dtype=np.float32).astype(np.float16)
    tgt = np.asarray(targetimg_fea, dtype=np.float32).astype(np.float16)
    in_maps = []
    for core in range(N_CORES):
        b, hh = core // 2, core % 2
        ref_s = ref[b, :, HP * hh:HP * (hh + 1), :].reshape(C, PIX)
        tgt_s = tgt[b, :, HP * hh:HP * (hh + 1), :].reshape(C, PIX)
        in_maps.append({"inq": _prep_core(ref_s, tgt_s), "cst": cst})
    res = run_bass_kernel_spmd(nc, in_maps, core_ids=list(range(N_CORES)),
                               trace=trace)
    out = np.empty((B, H, W), np.float32)
    for core in range(N_CORES):
        b, hh = core // 2, core % 2
        nd = res.results[core]["out"]          # [8, 1600]: den rows 0-3, num 4-7
        pred = nd[4:8] / nd[0:4]               # [4, 1600]
        out[b, HP * hh:HP * (hh + 1), :] = pred.reshape(HP, W)
    return out, res


def kernel(refimg_fea, targetimg_fea, maxdisp):
    assert int(maxdisp) == D, f"kernel hardcodes maxdisp={D}, got {maxdisp}"
    out, _ = _run(refimg_fea, targetimg_fea)
    return out


# revision 31
# speedup vs baseline: 1.0259x; 1.0259x over previous
"""HSMNet cost-volume + disparity softmax-regression on 8 Trainium2 NeuronCores.

Reference computation (per batch b):
  cost[c,d,h,w] = |ref[c,h,w] - tgt[c,h,w-d]| for w>=d else 0
  cost_agg[d,h,w] = sum_c cost
  pred[h,w] = sum_d d * softmax_d(cost_agg)

Sharding: 8 cores = 4 batches x 2 h-halves (40 rows of 80 each). Each core
processes its [32, 40, 160] slice fully fused on-chip.

Host prep (layout only, no arithmetic): inputs are cast to fp16 and
replicated into 4 partition groups (partition = c + 32*j) with the shift j
baked into tgt via a 24-col front zero pad. On-chip, per eighth of the
pixel range (800 pixels):
  - one DVE tensor_tensor subtract with a 3D access pattern (disparity
    block dim stride +4 on tgt, stride 0 broadcast on ref) produces diffs
    for all 24 disparities: diff[c+32j, k, p] = ref[c,p] - tgt[c, p-4b-j],
    b = 5-k.
  - abs in place, split across DVE (uint16 bitand), ACT (Abs), GPSIMD
    (uint16 bitand) per env-tunable column split.
  - TensorE reduces channels with 0/1 weights into PSUM [24, 2x512], plus
    one extra accumulation matmul that adds -10000 where w < d (validity
    mask folded into the PE pass: [w<d] = sum_k [k<d]*[w==k]).
  - ACT Exp evacuates PSUM -> E[96, 1600] bf16 (rows 24q+d).
  - TensorE contracts E with [ones; d] weights -> den/num [8, 1600].
  - host divides num/den (invalid entries' terms vanish: exp(-1e4) = 0).
"""
import os
import sys
import threading

for _p in ("/opt/trn_rl_repo",):
    if os.path.isdir(_p) and _p not in sys.path:
        sys.path.insert(0, _p)

import numpy as np
import ml_dtypes

import concourse.bacc as bacc
import concourse.mybir as mybir
from concourse.tile import TileContext
from concourse.bass_utils import run_bass_kernel_spmd

dt = mybir.dt

# problem shape (hardcoded per spec)
B, C, H, W = 4, 32, 80, 160
D = 24
HP = H // 2            # rows per core
PIX = HP * W           # 6400 pixels per core
NB = D // 4            # 6 disparity blocks of 4
PAD = 24               # zero pad columns in front of tgtr
NE = 8                 # processing units (eighths of the pixel range)
EW = PIX // NE         # 800 pixels per eighth
QW = PIX // 4          # 1600 pixels per quarter (E column range)
N_CORES = 8

# abs column split within each [128, 4800] diff tile: [0:A) DVE bitand,
# [A:4800) ACT Abs. The last TAIL_FULL eighths run the abs fully on DVE so
# ACT owes nothing after the DVE stream ends.
ABS_DVE = int(os.environ.get("HSM_ABS_DVE", "2240"))
TAIL_FULL = int(os.environ.get("HSM_TAIL_FULL", "2"))
DIFF_BUFS = int(os.environ.get("HSM_DIFF_BUFS", "5"))
COST_BUFS = int(os.environ.get("HSM_COST_BUFS", "2"))


# packed per-eighth input block: cols [0, EW) = ref eighth, [EW, EW+PAD+EW)
# = tgt window [EW*e - PAD, EW*e + EW)
INW = EW + PAD + EW  # 1624
# packed consts (all fp16 container): lred | lnd(bf16 bits) | lmask | maskc
CONST_W = NB * D + 8 + D + EW  # 144+8+24+800 = 976


def _build_program():
    nc = bacc.Bacc("TRN2", target_bir_lowering=False)
    inq_h = nc.dram_tensor("inq", [128, NE * INW], dt.float16,
                           kind="ExternalInput")
    cst_h = nc.dram_tensor("cst", [128, CONST_W], dt.float16,
                           kind="ExternalInput")
    out_h = nc.dram_tensor("out", [8, 4 * 400], dt.float32, kind="ExternalOutput")

    with TileContext(nc) as tc:
        with tc.tile_pool(name="const", bufs=1) as cpool, \
             tc.tile_pool(name="inp", bufs=4) as ipool, \
             tc.tile_pool(name="diffp", bufs=DIFF_BUFS) as dpool, \
             tc.tile_pool(name="ep", bufs=1) as epool:
            cst_sb = cpool.tile([128, CONST_W], dt.float16)
            lred_sb = cst_sb[:, 0:NB * D]
            lnd_sb = cst_sb[:, NB * D:NB * D + 8].bitcast(dt.bfloat16)
            lmask_sb = cst_sb[0:D, NB * D + 8:NB * D + 8 + D]
            maskc_sb = cst_sb[0:D, NB * D + 8 + D:CONST_W]

            E = epool.tile([128, QW], dt.bfloat16)

            # one packed DMA per eighth (ref + shifted tgt window), spread
            # over the sync/scalar/gpsimd trigger queues so the ~0.65us
            # descriptor-gen per DMA doesn't serialize ahead of block 0
            inq = {}

            def emit_load(e, eng):
                t_ = ipool.tile([128, INW], dt.float16, tag="inq",
                                name=f"inq_{e}")
                eng.dma_start(t_[:], inq_h[:, INW * e:INW * (e + 1)])
                inq[e] = t_

            emit_load(0, nc.sync)
            emit_load(1, nc.scalar)
            emit_load(2, nc.sync)
            emit_load(3, nc.scalar)
            nc.sync.dma_start(cst_sb[:], cst_h[:])
            # rows 32q+24..32q+31 are never written by the exp evac; zero
            # them once so the num/den matmul sees 0 (their lnd weights are
            # 0, but garbage could be inf/nan)
            nc.gpsimd.memset(E[:], 0.0)
            for e in range(4, NE):
                emit_load(e, nc.gpsimd)

            diffs = {}

            def emit_tt(e):
                diff = dpool.tile([128, NB * EW], dt.float16, tag="diff",
                                  name=f"diff_{e}")
                out = diff[:].rearrange("p (k x) -> p k x", x=EW)
                in0 = inq[e][:, 0:EW].unsqueeze(1).broadcast_to(
                    [128, NB, EW])
                # tgt part starts at col EW; block k reads window cols
                # [EW+4+4k, +EW) -> diff slot k holds block b = 5-k
                in1 = inq[e][:, EW + 4:EW + 4 + EW].unsqueeze(
                    1).broadcast_to([128, NB, EW]).copy()
                in1.ap = in1.ap[:1] + (((4, NB)),) + in1.ap[2:]
                nc.vector.tensor_tensor(out, in0, in1,
                                        mybir.AluOpType.subtract)
                diffs[e] = diff

            def emit_abs_dve(e):
                diff = diffs[e]
                a = NB * EW if e >= NE - TAIL_FULL else ABS_DVE
                if a > 0:
                    du = diff[:, 0:a].bitcast(dt.uint16)
                    nc.vector.tensor_scalar(du, du, 0x7FFF, None,
                                            mybir.AluOpType.bitwise_and)

            def emit_abs_act(e):
                diff = diffs[e]
                a = NB * EW if e >= NE - TAIL_FULL else ABS_DVE
                if a < NB * EW:
                    nc.scalar.activation(diff[:, a:], diff[:, a:],
                                         mybir.ActivationFunctionType.Abs)

            costs = {}

            def emit_pe(e, qpool):
                diff = diffs[e]
                cost = qpool.tile([D, 1024], dt.float32, tag="cost",
                                  name=f"cost_{e}")
                for k in range(NB):
                    b = NB - 1 - k
                    for cc in range(2):
                        nc.tensor.matmul(
                            cost[:, 512 * cc:512 * cc + 400],
                            lred_sb[:, D * b:D * (b + 1)],
                            diff[:, EW * k + 400 * cc:EW * k + 400 * cc + 400],
                            start=(k == 0), stop=False)
                for cc in range(2):
                    nc.tensor.matmul(
                        cost[:, 512 * cc:512 * cc + 400],
                        lmask_sb[:],
                        maskc_sb[:, 400 * cc:400 * cc + 400],
                        start=False, stop=(cc == 1))
                costs[e] = cost

            def emit_exp(e):
                q, hh = e // 2, e % 2
                cost = costs[e]
                src = cost[:].rearrange("p (k x) -> p k x", x=512)[:, :, 0:400]
                dst = E[32 * q:32 * q + D,
                        EW * hh:EW * (hh + 1)].rearrange(
                            "p (k x) -> p k x", x=400)
                nc.scalar.activation(dst, src, mybir.ActivationFunctionType.Exp)
                del costs[e], diffs[e]

            with tc.tile_pool(name="cost", bufs=COST_BUFS, space="PSUM") as qpool, \
                 tc.tile_pool(name="nd", bufs=1, space="PSUM") as npool:
                nd = npool.tile([8, 2048], dt.float32)

                def emit_nd(half):
                    # partial num/den contraction over quarters 2h, 2h+1
                    # (PE operands cannot start at partition 96, so halves)
                    for cc in range(4):
                        nc.tensor.matmul(
                            nd[:, 512 * cc:512 * cc + 400],
                            lnd_sb[64 * half:64 * half + 64, :],
                            E[64 * half:64 * half + 64,
                              400 * cc:400 * (cc + 1)],
                            start=(half == 0), stop=(half == 1))

                for e in range(NE):
                    emit_tt(e)
                    emit_abs_act(e)
                    if e >= 1:
                        emit_abs_dve(e - 1)
                        emit_pe(e - 1, qpool)
                        emit_exp(e - 1)
                        if e == 5:
                            emit_nd(0)
                emit_abs_dve(NE - 1)
                emit_pe(NE - 1, qpool)
                emit_exp(NE - 1)
                emit_nd(1)

                ndv = nd[:].rearrange("p (k x) -> p k x", x=512)[:, :, 0:400]
                out_sb = epool.tile([8, 4 * 400], dt.float32)
                outv = out_sb[:].rearrange("p (k x) -> p k x", x=400)
                nc.scalar.activation(outv[:, 0:2], ndv[:, 0:2],
                                     mybir.ActivationFunctionType.Copy)
                nc.sync.dma_start(out_h[:, 0:800], out_sb[:, 0:800])
                nc.scalar.activation(outv[:, 2:4], ndv[:, 2:4],
                                     mybir.ActivationFunctionType.Copy)
                nc.scalar.dma_start(out_h[:, 800:1600], out_sb[:, 800:1600])

    nc.compile()
    return nc


def _host_constants():
    """Single packed [128, CONST_W] fp16 tensor: lred | lnd bits | lmask | maskc."""
    cst = np.zeros((128, CONST_W), np.float16)
    # lred: block b sums channels of partition group j into cost row 4b+j
    for b in range(NB):
        for j in range(4):
            for c in range(C):
                cst[c + 32 * j, D * b + 4 * b + j] = 1.0

    # lnd (bf16, stored as raw bits in the fp16 container)
    lnd = np.zeros((128, 8), np.float32)
    for q in range(4):
        for d in range(D):
            lnd[d + 32 * q, q] = 1.0      # den
            lnd[d + 32 * q, 4 + q] = d    # num
    lnd16 = lnd.astype(ml_dtypes.bfloat16).view(np.float16)
    cst[:, NB * D:NB * D + 8] = lnd16

    # bias[d, p] = sum_k lmask[k, d] * maskc[k, p] = -1e4 * [(p mod W) < d]
    o = NB * D + 8
    for k in range(D):
        for d in range(D):
            if k < d:
                cst[k, o + d] = 1.0
    o = NB * D + 8 + D
    for k in range(D):
        cst[k, o + np.arange(EW)[np.arange(EW) % W == k]] = -10000.0
    return cst


_lock = threading.Lock()
_cache = {}


def _get_program():
    with _lock:
        if "nc" not in _cache:
            _cache["nc"] = _build_program()
            _cache["consts"] = _host_constants()
        return _cache["nc"], _cache["consts"]


def _prep_core(ref_s, tgt_s):
    """ref_s, tgt_s: [32, 6400] fp16 -> packed [128, NE*INW] input."""
    inq = np.zeros((128, NE * INW), np.float16)
    for e in range(NE):
        c0 = EW * e
        inq[:, INW * e:INW * e + EW] = np.broadcast_to(
            ref_s[None, :, c0:c0 + EW], (4, C, EW)).reshape(128, EW)
        # tgt window [c0 - PAD, c0 + EW) with per-group shift j baked in:
        # col EW + s holds tgt[c, c0 - PAD + s - j]
        for j in range(4):
            lo = c0 - PAD - j
            src = tgt_s[:, max(lo, 0):c0 + EW - j]
            inq[32 * j:32 * j + 32,
                INW * e + EW + max(lo, 0) - lo:
                INW * e + EW + max(lo, 0) - lo + src.shape[1]] = src
    return inq


def _run(refimg_fea, targetimg_fea, trace=False):
    nc, cst = _get_program()
    ref = np.asarray(refimg_fea, dtype=np.float32).astype(np.float16)
    tgt = np.asarray(targetimg_fea, dtype=np.float32).astype(np.float16)
    in_maps = []
    for core in range(N_CORES):
        b, hh = core // 2, core % 2
        ref_s = ref[b, :, HP * hh:HP * (hh + 1), :].reshape(C, PIX)
        tgt_s = tgt[b, :, HP * hh:HP * (hh + 1), :].reshape(C, PIX)
        in_maps.append({"inq": _prep_core(ref_s, tgt_s), "cst": cst})
    res = run_bass_kernel_spmd(nc, in_maps, core_ids=list(range(N_CORES)),
                               trace=trace)
    out = np.empty((B, H, W), np.float32)
    for core in range(N_CORES):
        b, hh = core // 2, core % 2
        nd = res.results[core]["out"]          # [8, 1600]: den rows 0-3, num 4-7
        pred = nd[4:8] / nd[0:4]               # [4, 1600]
        out[b, HP * hh:HP * (hh + 1), :] = pred.reshape(HP, W)
    return out, res


def kernel(refimg_fea, targetimg_fea, maxdisp):
    assert int(maxdisp) == D, f"kernel hardcodes maxdisp={D}, got {maxdisp}"
    out, _ = _run(refimg_fea, targetimg_fea)
    return out
